# revision 31
# baseline (speedup 1.0000x reference)
"""Multi-head causal attention (d_model=768, 12 heads, seq 2048, batch 2) on
8 Trainium2 NeuronCores.

Sharding: tensor-parallel over heads x data-parallel over batch.
Core c handles batch b = c // 4 and heads [3*(c%4), 3*(c%4)+3).
Each core computes its 3 heads' attention plus its partial output
projection; the host sums the 4 partials per batch and adds the bias
(exact fp32 reduction).

On-device layout (everything f32r = TF32-like PE mode, ~1e-4 rel err,
bf16-speed at moving-dim >= 256):
  - x.T is pre-transposed on host -> [768, 2048] so QKV projections are
    plain matmuls: stationary = stacked per-head weights, moving = x.T.
  - Scores are computed transposed (S.T = K @ Q.T, t on partitions), so
    softmax needs no on-chip reductions at all: exp on ScalarE, and the
    row sums l come from an extra ones-column appended to V in the
    P.T @ [V|1] matmul.  Causal masking: lower-triangle t-blocks are
    simply not computed; diagonal 128x128 sub-blocks are zeroed after
    exp by one DVE multiply with a precomputed triangle mask each.
  - O.T accumulates in PSUM per (head, 512-wide q block); normalization
    by 1/l broadcasts l with a ones-matmul on PE and takes a fast
    approximate reciprocal on DVE.
  - Output projection contracts the 3 heads' O.T (stacked 128+64 rows)
    against host-presliced Wo columns, drip-fed between attention chunks.
  - All contraction dims are zero-padded to K=128: the PE activity
    monitor never promotes the clock out of the half-rate state for
    half-array (K=64) matmuls, which costs ~1.6x.
"""

import sys
import types

import numpy as np

import concourse.bass as bass
import concourse.tile as tile
from concourse import mybir, bacc
from concourse.bass_utils import run_bass_kernel_spmd
from concourse.masks import make_identity

# Register the axon NTFF profiling hook if the environment supports it, so
# running with BASS_TRACE=1 yields exec_time_ns instead of an import error.
try:
    import antenv.axon_hooks  # noqa: F401
except ImportError:
    try:
        from trn_agent_boot.trn_boot import _ntff_profile_via_ctypes

        _hook = _ntff_profile_via_ctypes("/opt/axon/libaxon_pjrt.so")
        _mod = types.ModuleType("antenv.axon_hooks")
        _mod.get_axon_ntff_profile_hook = lambda: _hook
        _mod.set_axon_ntff_profile_hook = lambda h: None
        sys.modules["antenv.axon_hooks"] = _mod
    except Exception:
        pass

F32 = mybir.dt.float32
F32R = mybir.dt.float32r

N_CORES = 8
B = 2
S = 2048
D = 768
H = 12
DK = 64
H_PER_CORE = 3  # 12 heads / 4 head-groups
NSUP = S // 512  # 4 q super-blocks of 512
NKCH = D // 128  # 6 contraction chunks
SCALE = 0.125  # 1/sqrt(64)

_CACHED_NC = None


def build_bass():
    nc = bacc.Bacc()
    xT = nc.declare_dram_parameter("xT", [D, S], F32R, isOutput=False)
    w_all = nc.declare_dram_parameter("w_all", [D, 576], F32R, isOutput=False)
    w2 = nc.declare_dram_parameter("w2", [256, D], F32R, isOutput=False)
    out = nc.declare_dram_parameter("out", [S, D], F32, isOutput=True)

    with tile.TileContext(nc) as tc:
        with (
            tc.tile_pool(name="persist", bufs=1) as pers,
            tc.tile_pool(name="ptpool", bufs=8) as ptpool,
            tc.tile_pool(name="norm", bufs=3) as norm,
            tc.tile_pool(name="stage", bufs=3) as stage,
            tc.tile_pool(name="ps_mm", bufs=5, space="PSUM") as ps,
            tc.tile_pool(name="ps_ot", bufs=2, space="PSUM") as ps_ot,
            tc.tile_pool(name="ps_rps", bufs=1, space="PSUM") as ps_rps,
        ):
            # ---- persistent SBUF tiles ----
            xT_sb = [pers.tile([128, S], F32R, tag=f"xt{k}", name=f"xt{k}") for k in range(NKCH)]
            w_sb = [pers.tile([128, 576], F32R, tag=f"w{k}", name=f"w{k}") for k in range(NKCH)]
            w2a_sb = pers.tile([128, D], F32R, tag="w2a")
            w2b_sb = pers.tile([128, D], F32R, tag="w2b")
            ident = pers.tile([128, 128], F32, tag="ident")
            ones_sb = pers.tile([1, 64], F32R, tag="ones")
            # mask[p, 128+c] = 1 if c >= p else 0; mask[p, 0:128] = 0
            cmask = pers.tile([128, 256], F32R, tag="cmask")
            q_sb = [pers.tile([128, S], F32R, tag=f"q{h}", name=f"q{h}") for h in range(H_PER_CORE)]
            k_sb = [pers.tile([128, S], F32R, tag=f"k{h}", name=f"k{h}") for h in range(H_PER_CORE)]
            vt_sb = [pers.tile([128, S], F32R, tag=f"vt{h}", name=f"vt{h}") for h in range(H_PER_CORE)]
            # V natural + ones column, per head: [t-partition, block, 65]
            vn_sb = [
                pers.tile([128, S // 128, 65], F32R, tag=f"vn{h}", name=f"vn{h}")
                for h in range(H_PER_CORE)
            ]
            ot01 = pers.tile([128, S], F32R, tag="ot01")  # heads 0,1 O.T stacked
            ot2 = pers.tile([128, S], F32R, tag="ot2")  # head 2 O.T (rows 64:128 zero)

            # ---- input DMAs + constants ----
            for k in range(NKCH):
                nc.sync.dma_start(out=w_sb[k], in_=w_all[128 * k : 128 * k + 128, :])
            for nt in range(S // 512):
                for k in range(NKCH):
                    nc.sync.dma_start(
                        out=xT_sb[k][:, bass.ts(nt, 512)],
                        in_=xT[128 * k : 128 * k + 128, bass.ts(nt, 512)],
                    )
            nc.sync.dma_start(out=w2a_sb, in_=w2[0:128, :])
            nc.sync.dma_start(out=w2b_sb, in_=w2[128:256, :])
            make_identity(nc, ident)
            nc.vector.memset(ones_sb.bitcast(F32), 1.0)
            # PE warm-up: dependency-free dummy matmuls on a never-written
            # f32 tile (result discarded) keep the HAM activity monitor busy
            # during the input DMA wait so QKV starts at full clock
            wsrc = pers.tile([128, 128], F32, tag="wsrc")
            nc.vector.memset(wsrc, 1.0)
            wps = ps.tile([128, 512], F32, tag="mm", name="wps")
            for wi in range(14):
                nc.tensor.matmul(
                    wps[:, 0:128],
                    wsrc,
                    wsrc,
                    start=(wi == 0),
                    stop=(wi == 13),
                    skip_group_check=True,
                )
            def zero_pad_f32r(region):
                # write f32r-typed zeros (fill always taken; junk never read
                # arithmetically, and the producer is f32r-typed for the
                # BIR verifier)
                nc.gpsimd.affine_select(
                    out=region,
                    in_=region,
                    pattern=[[1, region.free_size()]],
                    compare_op=mybir.AluOpType.is_ge,
                    fill=0.0,
                    base=-(region.free_size() + 1),
                    channel_multiplier=0,
                )

            # causal mask tile: all zeros, then 1.0 where j >= 128 + p
            # (affine_select keeps in_ where the predicate is TRUE and
            # writes fill where FALSE)
            zero_pad_f32r(cmask)
            nc.gpsimd.affine_select(
                out=cmask,
                in_=cmask,
                pattern=[[-1, 256]],
                compare_op=mybir.AluOpType.is_ge,
                fill=1.0,
                base=127,
                channel_multiplier=1,
            )
            for h in range(H_PER_CORE):
                zero_pad_f32r(q_sb[h][64:128, :])
                zero_pad_f32r(k_sb[h][64:128, :])
                zero_pad_f32r(vt_sb[h][64:128, :])
            zero_pad_f32r(ot2[64:128, :])
            for h in range(H_PER_CORE):
                nc.vector.memset(vn_sb[h].bitcast(F32)[:, :, 64:65], 1.0)

            # ---- QKV projection ----
            # w_all columns: [Q0 K0 | Q1 K1 | Q2 K2 | V0 V1 | V2]
            mchunks = [
                (0, 128, q_sb[0], k_sb[0]),
                (128, 128, q_sb[1], k_sb[1]),
                (256, 128, q_sb[2], k_sb[2]),
                (384, 128, vt_sb[0], vt_sb[1]),
                (512, 64, vt_sb[2], None),
            ]
            for nt in range(S // 512):
                ncols = bass.ts(nt, 512)
                for mcol, msz, dst_a, dst_b in mchunks:
                    pt = ps.tile([128, 512], F32, tag="mm")
                    for k in range(NKCH):
                        nc.tensor.matmul(
                            pt[:msz, :],
                            w_sb[k][:, mcol : mcol + msz],
                            xT_sb[k][:, ncols],
                            start=(k == 0),
                            stop=(k == NKCH - 1),
                        )
                    nc.vector.tensor_copy(dst_a[0:64, ncols], pt[0:64, :])
                    if dst_b is not None:
                        nc.scalar.copy(dst_b[0:64, ncols], pt[64:128, :])

            # ---- V transposes: vt [64, S] -> vn blocks [128, 64] ----
            for h in range(H_PER_CORE):
                for blk in range(S // 128):
                    ptr = ps.tile([128, 512], F32, tag="mm", name="ptr")[:, 0:128]
                    nc.tensor.transpose(
                        ptr,
                        vt_sb[h][:, bass.ts(blk, 128)].bitcast(F32),
                        ident,
                    )
                    if blk % 2 == 0:
                        nc.vector.tensor_copy(vn_sb[h][:, blk, 0:64], ptr[:, 0:64])
                    else:
                        nc.scalar.copy(vn_sb[h][:, blk, 0:64], ptr[:, 0:64])

            # pt pool slots must hold finite values before first use (the
            # masked multiply does junk*0 and NaN*0 would poison it)
            for pi in range(8):
                ptz = ptpool.tile([128, 512], F32R, tag="pt", name=f"ptz{pi}")
                zero_pad_f32r(ptz)

            # ---- attention + interleaved output projection ----
            # outproj work is queued per 128-row block and drip-fed between
            # attention chunks (one block per 4 chunks) so the exp stream on
            # ScalarE never pauses while the PE picks up the extra matmuls
            op_queue = []

            def emit_one_op(qb):
                qs = bass.ts(qb, 128)
                ostage = stage.tile([128, D], F32, tag="ostage")
                for ncol, nlen in ((0, 512), (512, 256)):
                    pp = ps.tile([128, 512], F32, tag="mm", name="pp")[:, 0:nlen]
                    nc.tensor.matmul(
                        pp,
                        ot01[:, qs],
                        w2a_sb[:, ncol : ncol + nlen],
                        start=True,
                        stop=False,
                    )
                    nc.tensor.matmul(
                        pp,
                        ot2[:, qs],
                        w2b_sb[:, ncol : ncol + nlen],
                        start=False,
                        stop=True,
                    )
                    nc.vector.tensor_copy(ostage[:, ncol : ncol + nlen], pp)
                nc.sync.dma_start(out=out[qs, :], in_=ostage)

            def pop_op():
                if op_queue:
                    emit_one_op(op_queue.pop(0))

            for sup in range(NSUP):
                # lagged by one super-block so the PE never waits on the
                # normalization chain of the block being projected
                if sup > 0:
                    op_queue.extend(range(4 * (sup - 1), 4 * sup))
                for h in range(H_PER_CORE):
                    otp = ps_ot.tile([65, 512], F32, tag="ot", name="otp")
                    nchunks = 4 * sup + 4
                    qbase = 512 * sup
                    for j in range(nchunks):
                        d = j - 4 * sup  # >=0 on diagonal sub-blocks
                        c0 = 128 * d if d >= 0 else 0
                        # pad N up to 256: f32r is 4 cyc/row below 256 moving
                        ce = min(c0, 256)
                        stp = ps.tile([128, 512], F32, tag="mm")
                        nc.tensor.matmul(
                            stp[:, ce:512],
                            k_sb[h][:, bass.ts(j, 128)],
                            q_sb[h][:, qbase + ce : qbase + 512],
                            start=True,
                            stop=True,
                        )
                        ptile = ptpool.tile([128, 512], F32R, tag="pt")
                        nc.scalar.activation(
                            out=ptile[:, c0:512],
                            in_=stp[:, c0:512],
                            func=mybir.ActivationFunctionType.Exp,
                            scale=SCALE,
                        )
                        if d >= 0:
                            # zero cols [ce, c0) (stale-but-finite values) and
                            # the above-diagonal of the triangular sub-block
                            # via one DVE multiply with the precomputed mask
                            w = c0 + 128 - ce
                            nc.vector.tensor_mul(
                                ptile[:, ce : c0 + 128],
                                ptile[:, ce : c0 + 128],
                                cmask[:, 256 - w : 256],
                            )
                        nc.tensor.matmul(
                            otp[:, ce:512],
                            vn_sb[h][:, j, :],
                            ptile[:, ce:512],
                            start=(j == 0),
                            stop=(j == nchunks - 1),
                            skip_group_check=True,
                        )
                        if j % 4 == 3:
                            pop_op()
                    # normalize by 1/l: copy l to SBUF (f32r), broadcast
                    # across 64 partitions with a tiny ones-matmul on PE,
                    # then a fast approximate reciprocal on DVE (64 lanes)
                    lt = norm.tile([1, 512], F32R, tag="lt")
                    nc.vector.tensor_copy(lt, otp[64:65, :])
                    rps = ps_rps.tile([64, 512], F32, tag="rps", name="rps")
                    nc.tensor.matmul(rps, ones_sb, lt, start=True, stop=True)
                    rbc = norm.tile([64, 512], F32, tag="rbc")
                    nc.vector.reciprocal_approx_fast(out=rbc, in_=rps)
                    if h == 0:
                        dst = ot01[0:64, bass.ts(sup, 512)]
                    elif h == 1:
                        dst = ot01[64:128, bass.ts(sup, 512)]
                    else:
                        dst = ot2[0:64, bass.ts(sup, 512)]
                    nc.vector.tensor_mul(dst, otp[0:64, :], rbc)

            for qb in range(4 * (NSUP - 1), 4 * NSUP):
                emit_one_op(qb)

    nc.compile()
    return nc


def _get_nc():
    global _CACHED_NC
    if _CACHED_NC is None:
        _CACHED_NC = build_bass()
    return _CACHED_NC


def make_in_maps(x, Wq, Wk, Wv, Wo):
    x = np.asarray(x, dtype=np.float32)
    Wq = np.asarray(Wq, dtype=np.float32)
    Wk = np.asarray(Wk, dtype=np.float32)
    Wv = np.asarray(Wv, dtype=np.float32)
    Wo = np.asarray(Wo, dtype=np.float32)
    in_maps = []
    for c in range(N_CORES):
        b = c // 4
        hs = [H_PER_CORE * (c % 4) + i for i in range(H_PER_CORE)]
        xT_host = np.ascontiguousarray(x[b].T)  # [768, 2048]
        w_cols = []
        for h in hs:
            w_cols += [Wq[h], Wk[h]]
        for h in hs:
            w_cols.append(Wv[h])
        w_all = np.ascontiguousarray(np.concatenate(w_cols, axis=1))  # [768, 576]
        w2 = np.zeros((256, D), dtype=np.float32)
        w2[0:192] = np.concatenate([Wo[:, DK * h : DK * h + DK].T for h in hs], axis=0)
        in_maps.append({"xT": xT_host, "w_all": w_all, "w2": w2})
    return in_maps


def run_cores(in_maps, **kwargs):
    nc = _get_nc()
    return run_bass_kernel_spmd(nc, in_maps, core_ids=list(range(N_CORES)), **kwargs)


def kernel(x, Wq, Wk, Wv, Wo, bo):
    in_maps = make_in_maps(x, Wq, Wk, Wv, Wo)
    res = run_cores(in_maps)
    bo = np.asarray(bo, dtype=np.float32)
    out = np.empty((B, S, D), dtype=np.float32)
    for b in range(B):
        acc = res.results[4 * b]["out"].astype(np.float32)
        for c in range(4 * b + 1, 4 * b + 4):
            acc = acc + res.results[c]["out"]
        out[b] = acc + bo[None, :]
    return out
